# revision 23
# baseline (speedup 1.0000x reference)
"""Trainium2 Bass kernel for CausalTensionGraphLayer.

Math (u-fused factorization; W = 4, H = D/2):
  a   = x @ w1[:D] + b1                        [T, H]
  c   = x @ w1[D:]                             [T, H]   (shifted grid)
  u   = x @ (wv_w @ merge_w[D:]) + wv_b @ merge_w[D:]   [T, D] (shifted grid)
  hid_w  = silu(a[t] + c[t-w-1])
  tau_w  = sigmoid(hid_w @ w2 + b2) = 0.5 + 0.5*tanh(0.5*(...))
  mm[t]  = sum_w tau_w[t] * u[t-w-1]
  y      = x @ merge_w[:D] + mm + merge_b
  out    = LayerNorm(y) * gamma + beta

Fusing wv into merge_w[D:] on the host removes the whole msg @ m2 matmul
(the transposed mm is accumulated into the y PSUM with cheap 128-col
identity matmuls instead) and removes one 2.1 MB weight load.

The gating path (a, c, tau logits) runs in fp8-e4m3 with DoubleRow
matmuls (2x PE throughput); weights are pre-scaled by 32 on the host to
stay out of the fp8 subnormal range and the 1/32 is folded into the PSUM
eviction scale.  The value/merge path stays bf16 (it feeds y directly).
Measured end-to-end rel err ~5e-3 vs the fp32 reference.

Sharding: data-parallel over the B*T = 8192 token rows, 1024 own tokens
per core plus a 4-row halo (zeros at batch boundaries).  No collectives.

All device inputs are host-packed into the exact SBUF layout so every
input DMA is 128 fully contiguous rows (descriptor generation serialized
the old input stage).  The activation table is silu_and_others, so one
ACT_TABLE_LOAD serves the whole kernel; tau's sigmoid is computed as
0.5 + 0.5*tanh(x/2) with the affine folded into a cheap DVE
tensor_scalar.

Emission order software-pipelines quarters: AB0 AB1 C0 AB2 C1 D0 AB3 C2
D1 C3 D2 D3, so the PE stream never waits on the vector/scalar gating
chain.
"""

from contextlib import ExitStack

import numpy as np
import ml_dtypes

import concourse.bass as bass
import concourse.bacc as bacc
import concourse.tile as tile
from concourse import mybir
from concourse.bass_utils import run_bass_kernel_spmd

BF16 = ml_dtypes.bfloat16
F8 = ml_dtypes.float8_e4m3fn
W8SCALE = 32.0

B, T, D = 2, 4096, 1024
H = D // 2
W = 4
EPS = 1e-5
NCORES = 8
NTOK = (B * T) // NCORES          # 1024 own tokens per core
HALO = W                          # 4
GRID = NTOK + HALO                # 1028
NQ = 4                            # token quarters per core
QT = NTOK // NQ                   # 256 own tokens per quarter
QG = QT + HALO                    # 260 shifted-grid cols per quarter
KD = D // 128                     # 8 K-chunks over D
MH = H // 128                     # 4 M-tiles over H
MD = D // 128                     # 8 M-tiles over D
NT = QT // 128                    # 2 token tiles per quarter
G0 = 520                          # xT0 covers grid [0, 520), xT1 [512, 1028)
G1 = GRID - 512                   # 516

FP32 = mybir.dt.float32
I32 = mybir.dt.int32
BF = mybir.dt.bfloat16
E4 = mybir.dt.float8e4
AF = mybir.ActivationFunctionType
ALU = mybir.AluOpType
AX = mybir.AxisListType
DR = mybir.MatmulPerfMode.DoubleRow


def build_nc(use_gamma_beta, use_merge_b, use_b1, use_b2, use_ub):
    nc = bacc.Bacc(None, target_bir_lowering=False)

    xT0 = nc.dram_tensor("xT0", [128, KD * G0], BF, kind="ExternalInput")
    xT1 = nc.dram_tensor("xT1", [128, KD * G1], BF, kind="ExternalInput")
    x80 = nc.dram_tensor("x80", [128, KD * G0], E4, kind="ExternalInput")
    x81 = nc.dram_tensor("x81", [128, KD * G1], E4, kind="ExternalInput")
    w1a8 = nc.dram_tensor("w1a8", [128, KD * H], E4, kind="ExternalInput")
    w1c8 = nc.dram_tensor("w1c8", [128, KD * H], E4, kind="ExternalInput")
    wfA = nc.dram_tensor("wfA", [128, KD * 512], BF, kind="ExternalInput")
    wfB = nc.dram_tensor("wfB", [128, KD * 512], BF, kind="ExternalInput")
    m1h0 = nc.dram_tensor("m1h0", [128, KD * 512], BF, kind="ExternalInput")
    m1h1 = nc.dram_tensor("m1h1", [128, KD * 512], BF, kind="ExternalInput")
    w2r8 = nc.dram_tensor("w2r8", [128, MH * 128], E4, kind="ExternalInput")
    idd = nc.dram_tensor("idd", [128, 128], BF, kind="ExternalInput")
    if use_b1:
        b1r = nc.dram_tensor("b1r", [128, MH], FP32, kind="ExternalInput")
    if use_ub:
        ubr = nc.dram_tensor("ubr", [128, MD], FP32, kind="ExternalInput")
    if use_b2:
        b2h = nc.dram_tensor("b2h", [128, 1], FP32, kind="ExternalInput")
    if use_gamma_beta:
        gam = nc.dram_tensor("gam", [1, D], FP32, kind="ExternalInput")
        bet = nc.dram_tensor("bet", [1, D], FP32, kind="ExternalInput")
    if use_merge_b:
        mbt = nc.dram_tensor("mbt", [1, D], FP32, kind="ExternalInput")
    y = nc.dram_tensor("y", [NTOK, D], BF, kind="ExternalOutput")

    with tile.TileContext(nc) as tc, ExitStack() as ctx:
        persist = ctx.enter_context(tc.tile_pool(name="persist", bufs=1))
        abpool = ctx.enter_context(tc.tile_pool(name="abpool", bufs=NQ))
        qpool = ctx.enter_context(tc.tile_pool(name="qpool", bufs=2))
        mpool = ctx.enter_context(tc.tile_pool(name="mpool", bufs=4))
        mpool2 = ctx.enter_context(tc.tile_pool(name="mpool2", bufs=2))
        opool = ctx.enter_context(tc.tile_pool(name="opool", bufs=5))
        ps_acc = ctx.enter_context(tc.tile_pool(name="ps_acc", bufs=3, space="PSUM"))
        ps_log = ctx.enter_context(tc.tile_pool(name="ps_log", bufs=1, space="PSUM"))
        ps_y = ctx.enter_context(tc.tile_pool(name="ps_y", bufs=4, space="PSUM"))

        # ---- persistent SBUF tiles -------------------------------------
        xT0_sb = persist.tile([128, KD, G0], BF, tag="xT0")
        xT1_sb = persist.tile([128, KD, G1], BF, tag="xT1")
        x80_sb = persist.tile([128, KD, G0], E4, tag="x80")
        x81_sb = persist.tile([128, KD, G1], E4, tag="x81")
        w1a_sb = persist.tile([128, KD, H], E4, tag="w1a8")
        w1c_sb = persist.tile([128, KD, H], E4, tag="w1c8")
        wf_sb = persist.tile([128, KD, D], BF, tag="wf")
        m1h0_sb = persist.tile([128, KD, 512], BF, tag="m1h0")
        m1h1_sb = persist.tile([128, KD, 512], BF, tag="m1h1")
        m1_sb = [m1h0_sb, m1h1_sb]
        w2r_sb = persist.tile([128, MH, 128], E4, tag="w2r8")
        id_sb = persist.tile([128, 128], BF, tag="idd")

        # ---- input DMAs: all host-packed contiguous [128, bytes] -------
        # The fp8 x half and the fp8 gating weights land first on separate
        # queues so phase A can start ~10 us in.
        nc.sync.dma_start(out=x80_sb, in_=x80[:, :])
        nc.scalar.dma_start(out=w1a_sb, in_=w1a8[:, :])
        nc.scalar.dma_start(out=x81_sb, in_=x81[:, :])
        nc.scalar.dma_start(out=w1c_sb, in_=w1c8[:, :])
        nc.scalar.dma_start(out=wf_sb[:, :, 0:512], in_=wfA[:, :])
        nc.scalar.dma_start(out=wf_sb[:, :, 512:D], in_=wfB[:, :])
        nc.scalar.dma_start(out=w2r_sb, in_=w2r8[:, :])
        nc.scalar.dma_start(out=id_sb, in_=idd[:, :])
        if use_b1:
            b1_sb = persist.tile([128, MH], FP32, tag="b1")
            nc.scalar.dma_start(out=b1_sb, in_=b1r[:, :])
        if use_ub:
            ub_sb = persist.tile([128, MD], FP32, tag="ub")
            nc.scalar.dma_start(out=ub_sb, in_=ubr[:, :])
        if use_b2:
            b2_sb = persist.tile([128, 1], FP32, tag="b2")
            nc.scalar.dma_start(out=b2_sb, in_=b2h[:, :])
        nc.sync.dma_start(out=xT0_sb, in_=xT0[:, :])
        nc.sync.dma_start(out=xT1_sb, in_=xT1[:, :])
        nc.sync.dma_start(out=m1_sb[0], in_=m1h0[:, :])
        nc.sync.dma_start(out=m1_sb[1], in_=m1h1[:, :])
        if use_gamma_beta:
            gam_sb = persist.tile([128, D], FP32, tag="gam")
            nc.sync.dma_start(out=gam_sb, in_=gam.partition_broadcast(128))
            bet_sb = persist.tile([128, D], FP32, tag="bet")
            nc.sync.dma_start(out=bet_sb, in_=bet.partition_broadcast(128))
        if use_merge_b:
            mb_sb = persist.tile([128, D], FP32, tag="mb")
            nc.sync.dma_start(out=mb_sb, in_=mbt.partition_broadcast(128))

        magic_sb = persist.tile([128, 1], I32, tag="magic")
        nc.vector.memset(magic_sb, 0x5F3759DF)
        one_i = persist.tile([128, 1], I32, tag="onei")
        nc.vector.memset(one_i, 1)

        # quarter -> (bf16 x tile, fp8 x tile, shifted-grid base col)
        gmap = [
            (xT0_sb, x80_sb, 0), (xT0_sb, x80_sb, 256),
            (xT1_sb, x81_sb, 0), (xT1_sb, x81_sb, 256),
        ]
        ISCALE = 1.0 / W8SCALE

        aqs, cqs, uqs, tauqs, mmqs = {}, {}, {}, {}, {}

        def emit_A(q):
            xs, x8, base = gmap[q]
            aq = abpool.tile([128, MH, QT], BF, tag="aq")
            aqs[q] = aq
            for m in range(MH):
                ps = ps_acc.tile([128, QG], FP32, tag="acc")
                for kp in range(KD // 2):
                    nc.tensor.matmul(
                        ps[:, 0:QT],
                        w1a_sb[:, 2 * kp:2 * kp + 2, m * 128:(m + 1) * 128],
                        x8[:, 2 * kp:2 * kp + 2, base + HALO:base + HALO + QT],
                        start=(kp == 0), stop=(kp == KD // 2 - 1),
                        perf_mode=DR,
                    )
                if use_b1:
                    nc.scalar.activation(
                        out=aq[:, m, :], in_=ps[:, 0:QT], func=AF.Identity,
                        bias=b1_sb[:, m:m + 1], scale=ISCALE,
                    )
                elif m % 2 == 0:
                    nc.scalar.activation(
                        out=aq[:, m, :], in_=ps[:, 0:QT], func=AF.Identity,
                        bias=0.0, scale=ISCALE,
                    )
                else:
                    nc.vector.tensor_scalar_mul(aq[:, m, :], ps[:, 0:QT], ISCALE)

        def emit_CC(q):
            xs, x8, base = gmap[q]
            cq = abpool.tile([128, MH, QG], BF, tag="cq")
            cqs[q] = cq
            for m in range(MH):
                ps = ps_acc.tile([128, QG], FP32, tag="acc")
                for kp in range(KD // 2):
                    nc.tensor.matmul(
                        ps,
                        w1c_sb[:, 2 * kp:2 * kp + 2, m * 128:(m + 1) * 128],
                        x8[:, 2 * kp:2 * kp + 2, base:base + QG],
                        start=(kp == 0), stop=(kp == KD // 2 - 1),
                        perf_mode=DR,
                    )
                if m % 2 == 0:
                    nc.scalar.activation(
                        out=cq[:, m, :], in_=ps, func=AF.Identity,
                        bias=0.0, scale=ISCALE,
                    )
                else:
                    nc.vector.tensor_scalar_mul(cq[:, m, :], ps, ISCALE)
        def emit_U(q):
            xs, x8, base = gmap[q]
            uq = abpool.tile([128, MD, QG], BF, tag="uq")
            uqs[q] = uq
            for m in range(MD):
                ps = ps_acc.tile([128, QG], FP32, tag="acc")
                for k in range(KD):
                    nc.tensor.matmul(
                        ps, wf_sb[:, k, m * 128:(m + 1) * 128],
                        xs[:, k, base:base + QG],
                        start=(k == 0), stop=(k == KD - 1),
                    )
                if use_ub:
                    nc.scalar.activation(
                        out=uq[:, m, :], in_=ps, func=AF.Identity,
                        bias=ub_sb[:, m:m + 1], scale=1.0,
                    )
                else:
                    nc.scalar.copy(out=uq[:, m, :], in_=ps)

        def emit_C(q):
            aq, cq, uq = aqs[q], cqs[q], uqs[q]
            tauq = qpool.tile([128, W, QT], BF, tag="tauq")
            tauqs[q] = tauq

            def tau_b(w):
                s = tauq[:, w, :]
                return bass.AP(
                    tensor=s.tensor, offset=s.offset,
                    ap=[s.ap[0], [0, MD], s.ap[1]],
                )

            pw = {}
            m01 = None
            for p in range(W // 2):
                hs = mpool2.tile([128, MH, 2, QT], BF, tag="hs")
                for wi in range(2):
                    w = 2 * p + wi
                    o = HALO - 1 - w
                    nc.vector.tensor_add(hs[:, :, wi, :], aq, cq[:, :, o:o + QT])
                hss = mpool2.tile([128, MH, 2, QT], E4, tag="hss")
                nc.scalar.activation(out=hss, in_=hs, func=AF.Silu)
                pl = ps_log.tile([128, 2 * QT], FP32, tag="logit")
                for kp in range(MH // 2):
                    nc.tensor.matmul(
                        pl, w2r_sb[:, 2 * kp:2 * kp + 2, :],
                        hss[:, 2 * kp:2 * kp + 2, :, :],
                        start=(kp == 0), stop=(kp == MH // 2 - 1),
                        perf_mode=DR,
                    )
                # tau = 0.5 + 0.5*tanh(0.5*(logit + b2)); affine done on DVE
                nc.scalar.activation(
                    out=tauq[:, 2 * p:2 * p + 2, :],
                    in_=pl.rearrange("p (a b) -> p a b", a=2),
                    func=AF.Tanh,
                    bias=(b2_sb[:, 0:1] if use_b2 else 0.0),
                    scale=0.5 * ISCALE,
                )
                nc.vector.tensor_scalar(
                    out=tauq[:, 2 * p:2 * p + 2, :],
                    in0=tauq[:, 2 * p:2 * p + 2, :],
                    scalar1=0.5, scalar2=0.5, op0=ALU.mult, op1=ALU.add,
                )
                for wi in range(2):
                    w = 2 * p + wi
                    o = HALO - 1 - w
                    pt = mpool.tile([128, MD, QT], BF, tag="pw")
                    nc.vector.tensor_mul(pt, tau_b(w), uq[:, :, o:o + QT])
                    pw[w] = pt
                if p == 0:
                    m01 = mpool.tile([128, MD, QT], BF, tag="pw")
                    nc.vector.tensor_add(m01, pw[0], pw[1])
            mmq = qpool.tile([128, MD, QT], BF, tag="mmq")
            mmqs[q] = mmq
            nc.vector.tensor_add(pw[3], pw[2], pw[3])
            nc.vector.tensor_add(mmq, m01, pw[3])

        def ln_finalize(q, srow, sqs, ysb, tts):
            # LayerNorm stats for token tiles `tts`; rstd via bit-trick seed
            # + 1 Newton step
            n = len(tts)
            ssum = mpool.tile([128, n], FP32, tag="ssum")
            nc.vector.reduce_sum(out=ssum, in_=srow, axis=AX.X)
            qsum = mpool.tile([128, n], FP32, tag="qsum")
            nc.vector.reduce_sum(out=qsum, in_=sqs, axis=AX.X)
            mean = mpool.tile([128, n], FP32, tag="mean")
            nc.vector.tensor_scalar_mul(mean, ssum, 1.0 / D)
            m2e = mpool.tile([128, n], FP32, tag="m2e")
            nc.vector.scalar_tensor_tensor(   # mean^2 - eps
                out=m2e, in0=mean, scalar=1.0, in1=mean,
                op0=ALU.mult, op1=ALU.mult,
            )
            nc.vector.tensor_scalar_add(m2e, m2e, -EPS)
            veps = mpool.tile([128, n], FP32, tag="veps")
            nc.vector.scalar_tensor_tensor(   # q/D - (mean^2 - eps)
                out=veps, in0=qsum, scalar=1.0 / D, in1=m2e,
                op0=ALU.mult, op1=ALU.subtract,
            )
            rbits = mpool.tile([128, n], I32, tag="rbits")
            nc.vector.tensor_scalar(
                out=rbits, in0=veps.bitcast(I32), scalar1=one_i[:, 0:1],
                scalar2=None, op0=ALU.arith_shift_right,
            )
            nc.vector.tensor_tensor(
                out=rbits, in0=magic_sb.to_broadcast([128, n]), in1=rbits,
                op=ALU.subtract,
            )
            rstd = rbits.bitcast(FP32)
            nt1 = mpool.tile([128, n], FP32, tag="nt1")
            nc.vector.tensor_mul(nt1, rstd, rstd)
            nc.vector.tensor_mul(nt1, nt1, veps)
            nc.vector.tensor_scalar(
                out=nt1, in0=nt1, scalar1=-0.5, scalar2=1.5,
                op0=ALU.mult, op1=ALU.add,
            )
            nc.vector.tensor_mul(rstd, rstd, nt1)
            for i, tt in enumerate(tts):
                tok0 = q * QT + 128 * tt
                nc.vector.tensor_scalar(
                    out=ysb[i], in0=ysb[i], scalar1=mean[:, i:i + 1],
                    scalar2=rstd[:, i:i + 1],
                    op0=ALU.subtract, op1=ALU.mult,
                )
                if use_gamma_beta:
                    nc.vector.tensor_mul(ysb[i], ysb[i], gam_sb)
                    nc.vector.tensor_add(ysb[i], ysb[i], bet_sb)
                nc.sync.dma_start(out=y[tok0:tok0 + 128, :], in_=ysb[i])

        def emit_evict(q, tt, yps, yt, srow, sqs):
            for half in range(2):
                n0 = half * 512
                if use_merge_b:
                    nc.vector.tensor_add(
                        yps[half], yps[half], mb_sb[:, n0:n0 + 512]
                    )
                nc.scalar.activation(
                    out=yt[:, n0:n0 + 512], in_=yps[half], func=AF.Copy,
                    accum_out=srow[:, 0, half:half + 1],
                )
                y2 = mpool2.tile([128, 512], BF, tag="y2")
                nc.vector.scalar_tensor_tensor(
                    out=y2, in0=yt[:, n0:n0 + 512], scalar=1.0,
                    in1=yt[:, n0:n0 + 512], op0=ALU.mult, op1=ALU.mult,
                    accum_out=sqs[:, 0, half:half + 1],
                )

        def emit_D(q, last=False):
            xs, _, base = gmap[q]
            mmq = mmqs[q]
            if not last:
                # m1 matmuls for both token tiles first (they do not depend
                # on the gating chain), then the mm transposes, then evicts.
                ypss, yts, srows, sqss = [], [], [], []
                for tt in range(NT):
                    tcol = base + HALO + tt * 128
                    yp0 = ps_y.tile([128, 512], FP32, tag="y")
                    yp1 = ps_y.tile([128, 512], FP32, tag="y")
                    ypss.append([yp0, yp1])
                    for k in range(KD):
                        for half in range(2):
                            nc.tensor.matmul(
                                ypss[tt][half], xs[:, k, tcol:tcol + 128],
                                m1_sb[half][:, k, :],
                                start=(k == 0), stop=False,
                            )
                for tt in range(NT):
                    for m in range(MD):
                        half, j0 = m // 4, (m % 4) * 128
                        nc.tensor.matmul(
                            ypss[tt][half][:, j0:j0 + 128],
                            mmq[:, m, tt * 128:tt * 128 + 128], id_sb,
                            start=False, stop=(m % 4 == 3),
                            skip_group_check=True,
                        )
                for tt in range(NT):
                    yt = opool.tile([128, D], BF, tag="ysb")
                    srow = mpool.tile([128, 1, 2], FP32, tag="srow")
                    sqs = mpool.tile([128, 1, 2], FP32, tag="sqs")
                    yts.append(yt)
                    srows.append(srow)
                    sqss.append(sqs)
                    emit_evict(q, tt, ypss[tt], yt, srow, sqs)
                for tt in range(NT):
                    ln_finalize(q, srows[tt], sqss[tt], [yts[tt]], [tt])
            else:
                # last quarter: token tiles fully sequential so the tail
                # chain after the final matmul is as short as possible
                for tt in range(NT):
                    tcol = base + HALO + tt * 128
                    yt = opool.tile([128, D], BF, tag="ysb")
                    srow = mpool.tile([128, 1, 2], FP32, tag="srow")
                    sqs = mpool.tile([128, 1, 2], FP32, tag="sqs")
                    yp0 = ps_y.tile([128, 512], FP32, tag="y")
                    yp1 = ps_y.tile([128, 512], FP32, tag="y")
                    yps = [yp0, yp1]
                    for k in range(KD):
                        for half in range(2):
                            nc.tensor.matmul(
                                yps[half], xs[:, k, tcol:tcol + 128],
                                m1_sb[half][:, k, :],
                                start=(k == 0), stop=False,
                            )
                    for m in range(MD):
                        half, j0 = m // 4, (m % 4) * 128
                        nc.tensor.matmul(
                            yps[half][:, j0:j0 + 128],
                            mmq[:, m, tt * 128:tt * 128 + 128], id_sb,
                            start=False, stop=(m % 4 == 3),
                            skip_group_check=True,
                        )
                    emit_evict(q, tt, yps, yt, srow, sqs)
                    ln_finalize(q, srow, sqs, [yt], [tt])

        # software-pipelined emission: a+c for all quarters first (they
        # only need the small fp8 inputs, so the PE starts ~10 us in and
        # stays busy while the bf16 x/weights stream); D lags C by one
        # quarter.  Each phase gets a monotone sim-time floor
        # (tile_wait_until) so the list scheduler cannot hoist a
        # later-phase instruction (whose input DMA is still in flight on
        # hardware) ahead of ready work in the in-order engine queues --
        # without the floors it parked a U-phase matmul mid-AC, stalling
        # the PE ~8 us on the xT0 transfer.
        phases = [
            (lambda q=q: emit_A(q), 0.001 * q) for q in range(NQ)
        ] + [
            (lambda q=q: emit_CC(q), 0.004 + 0.001 * q) for q in range(NQ)
        ] + [
            (lambda: emit_U(0), 0.008),
            (lambda: emit_U(1), 0.010),
            (lambda: emit_C(0), 0.010),
            (lambda: emit_U(2), 0.012),
            (lambda: emit_C(1), 0.012),
            (lambda: emit_D(0), 0.014),
            (lambda: emit_U(3), 0.016),
            (lambda: emit_C(2), 0.016),
            (lambda: emit_D(1), 0.018),
            (lambda: emit_C(3), 0.020),
            (lambda: emit_D(2), 0.020),
            (lambda: emit_D(3, last=True), 0.022),
        ]
        for fn, floor_ms in phases:
            with tc.tile_wait_until(floor_ms):
                fn()
    nc.compile()
    return nc


_CACHE: dict = {}


def _get_nc(*flags):
    if flags not in _CACHE:
        _CACHE[flags] = build_nc(*flags)
    return _CACHE[flags]


def _pack(a):
    # [D, F] -> [128, KD*F] in the SBUF layout (partition = d % 128 within
    # each 128-row K-chunk)
    d, f = a.shape
    return np.ascontiguousarray(
        a.reshape(d // 128, 128, f).transpose(1, 0, 2).reshape(128, -1)
    )


def kernel(x, w1, b1, w2, b2, wv_w, wv_b, merge_w, merge_b, gamma, beta):
    x = np.asarray(x, dtype=np.float32)
    w1 = np.asarray(w1, dtype=np.float32)
    b1 = np.asarray(b1, dtype=np.float32)
    w2 = np.asarray(w2, dtype=np.float32)
    b2 = np.asarray(b2, dtype=np.float32)
    wv_w = np.asarray(wv_w, dtype=np.float32)
    wv_b = np.asarray(wv_b, dtype=np.float32)
    merge_w = np.asarray(merge_w, dtype=np.float32)
    merge_b = np.asarray(merge_b, dtype=np.float32)
    gamma = np.asarray(gamma, dtype=np.float32)
    beta = np.asarray(beta, dtype=np.float32)

    m2 = merge_w[D:]
    wfuse = wv_w @ m2
    ubias = wv_b @ m2
    use_gamma_beta = not (np.all(gamma == 1.0) and np.all(beta == 0.0))
    use_merge_b = bool(np.any(merge_b != 0.0))
    use_b1 = bool(np.any(b1 != 0.0))
    use_b2 = bool(np.any(b2 != 0.0))
    use_ub = bool(np.any(ubias != 0.0))
    nc = _get_nc(use_gamma_beta, use_merge_b, use_b1, use_b2, use_ub)

    shared = {
        "w1a8": _pack((W8SCALE * w1[:D]).astype(F8)),
        "w1c8": _pack((W8SCALE * w1[D:]).astype(F8)),
        "wfA": _pack(wfuse[:, 0:512].astype(BF16)),
        "wfB": _pack(wfuse[:, 512:D].astype(BF16)),
        "m1h0": _pack(merge_w[:D, 0:512].astype(BF16)),
        "m1h1": _pack(merge_w[:D, 512:D].astype(BF16)),
        "w2r8": _pack(
            np.ascontiguousarray(
                np.broadcast_to(
                    (W8SCALE * w2).reshape(H, 1), (H, 128)
                )
            ).astype(F8)
        ),
        "idd": np.eye(128, dtype=np.float32).astype(BF16),
    }
    if use_b1:
        shared["b1r"] = np.ascontiguousarray(b1.reshape(MH, 128).T)
    if use_ub:
        shared["ubr"] = np.ascontiguousarray(ubias.reshape(MD, 128).T)
    if use_b2:
        shared["b2h"] = np.full((128, 1), 0.5 * float(b2[0]), np.float32)
    if use_gamma_beta:
        shared["gam"] = gamma.reshape(1, D)
        shared["bet"] = beta.reshape(1, D)
    if use_merge_b:
        shared["mbt"] = merge_b.reshape(1, D)

    x2 = x.reshape(B * T, D)
    in_maps = []
    for c in range(NCORES):
        t0 = c * NTOK
        xs = np.zeros((GRID, D), np.float32)
        xs[HALO:] = x2[t0:t0 + NTOK]
        if t0 % T != 0:  # halo stays inside the same batch element
            xs[:HALO] = x2[t0 - HALO:t0]
        xt = np.ascontiguousarray(xs.T).astype(BF16)
        x8full = xt.astype(np.float32).astype(F8)
        m = dict(shared)
        m["xT0"] = _pack(xt[:, 0:G0])
        m["xT1"] = _pack(xt[:, 512:GRID])
        m["x80"] = _pack(x8full[:, 0:G0])
        m["x81"] = _pack(x8full[:, 512:GRID])
        in_maps.append(m)

    res = run_bass_kernel_spmd(nc, in_maps, core_ids=list(range(NCORES)))
    out = np.concatenate(
        [r["y"].astype(np.float32) for r in res.results], axis=0
    )
    return out.reshape(B, T, D)


# revision 24
# speedup vs baseline: 1.0691x; 1.0691x over previous
"""Trainium2 Bass kernel for CausalTensionGraphLayer.

Math (u-fused factorization; W = 4, H = D/2):
  a   = x @ w1[:D] + b1                        [T, H]
  c   = x @ w1[D:]                             [T, H]   (shifted grid)
  u   = x @ (wv_w @ merge_w[D:]) + wv_b @ merge_w[D:]   [T, D] (shifted grid)
  hid_w  = silu(a[t] + c[t-w-1])
  tau_w  = sigmoid(hid_w @ w2 + b2) = 0.5 + 0.5*tanh(0.5*(...))
  mm[t]  = sum_w tau_w[t] * u[t-w-1]
  y      = x @ merge_w[:D] + mm + merge_b
  out    = LayerNorm(y) * gamma + beta

Fusing wv into merge_w[D:] on the host removes the whole msg @ m2 matmul
(the transposed mm is accumulated into the y PSUM with cheap 128-col
identity matmuls instead) and removes one 2.1 MB weight load.

The gating path (a, c, tau logits) runs in fp8-e4m3 with DoubleRow
matmuls (2x PE throughput); weights are pre-scaled by 32 on the host to
stay out of the fp8 subnormal range and the 1/32 is folded into the PSUM
eviction scale.  The value/merge path stays bf16 (it feeds y directly).
Measured end-to-end rel err ~5e-3 vs the fp32 reference.

Sharding: data-parallel over the B*T = 8192 token rows, 1024 own tokens
per core plus a 4-row halo (zeros at batch boundaries).  No collectives.

All device inputs are host-packed into the exact SBUF layout so every
input DMA is 128 fully contiguous rows (descriptor generation serialized
the old input stage).  The activation table is silu_and_others, so one
ACT_TABLE_LOAD serves the whole kernel; tau's sigmoid is computed as
0.5 + 0.5*tanh(x/2) with the affine folded into a cheap DVE
tensor_scalar.

Emission order software-pipelines quarters: AB0 AB1 C0 AB2 C1 D0 AB3 C2
D1 C3 D2 D3, so the PE stream never waits on the vector/scalar gating
chain.
"""

from contextlib import ExitStack

import numpy as np
import ml_dtypes

import concourse.bass as bass
import concourse.bacc as bacc
import concourse.tile as tile
from concourse import mybir
from concourse.bass_utils import run_bass_kernel_spmd

BF16 = ml_dtypes.bfloat16
F8 = ml_dtypes.float8_e4m3fn
W8SCALE = 32.0

B, T, D = 2, 4096, 1024
H = D // 2
W = 4
EPS = 1e-5
NCORES = 8
NTOK = (B * T) // NCORES          # 1024 own tokens per core
HALO = W                          # 4
GRID = NTOK + HALO                # 1028
NQ = 4                            # token quarters per core
QT = NTOK // NQ                   # 256 own tokens per quarter
QG = QT + HALO                    # 260 shifted-grid cols per quarter
KD = D // 128                     # 8 K-chunks over D
MH = H // 128                     # 4 M-tiles over H
MD = D // 128                     # 8 M-tiles over D
NT = QT // 128                    # 2 token tiles per quarter
G0 = 520                          # xT0 covers grid [0, 520), xT1 [512, 1028)
G1 = GRID - 512                   # 516

FP32 = mybir.dt.float32
I32 = mybir.dt.int32
BF = mybir.dt.bfloat16
E4 = mybir.dt.float8e4
AF = mybir.ActivationFunctionType
ALU = mybir.AluOpType
AX = mybir.AxisListType
DR = mybir.MatmulPerfMode.DoubleRow


def build_nc(use_gamma_beta, use_merge_b, use_b1, use_b2, use_ub):
    nc = bacc.Bacc(None, target_bir_lowering=False)

    xT0 = nc.dram_tensor("xT0", [128, KD * G0], BF, kind="ExternalInput")
    xT1 = nc.dram_tensor("xT1", [128, KD * G1], BF, kind="ExternalInput")
    x80 = nc.dram_tensor("x80", [128, KD * G0], E4, kind="ExternalInput")
    x81 = nc.dram_tensor("x81", [128, KD * G1], E4, kind="ExternalInput")
    w1a8 = nc.dram_tensor("w1a8", [128, KD * H], E4, kind="ExternalInput")
    w1c8 = nc.dram_tensor("w1c8", [128, KD * H], E4, kind="ExternalInput")
    wfA = nc.dram_tensor("wfA", [128, KD * 512], BF, kind="ExternalInput")
    wfB = nc.dram_tensor("wfB", [128, KD * 512], BF, kind="ExternalInput")
    m1h0 = nc.dram_tensor("m1h0", [128, KD * 512], BF, kind="ExternalInput")
    m1h1 = nc.dram_tensor("m1h1", [128, KD * 512], BF, kind="ExternalInput")
    w2r8 = nc.dram_tensor("w2r8", [128, MH * 128], E4, kind="ExternalInput")
    idd = nc.dram_tensor("idd", [128, 128], BF, kind="ExternalInput")
    if use_b1:
        b1r = nc.dram_tensor("b1r", [128, MH], FP32, kind="ExternalInput")
    if use_ub:
        ubr = nc.dram_tensor("ubr", [128, MD], FP32, kind="ExternalInput")
    if use_b2:
        b2h = nc.dram_tensor("b2h", [128, 1], FP32, kind="ExternalInput")
    if use_gamma_beta:
        gam = nc.dram_tensor("gam", [1, D], FP32, kind="ExternalInput")
        bet = nc.dram_tensor("bet", [1, D], FP32, kind="ExternalInput")
    if use_merge_b:
        mbt = nc.dram_tensor("mbt", [1, D], FP32, kind="ExternalInput")
    y = nc.dram_tensor("y", [NTOK, D], BF, kind="ExternalOutput")

    with tile.TileContext(nc) as tc, ExitStack() as ctx:
        persist = ctx.enter_context(tc.tile_pool(name="persist", bufs=1))
        abpool = ctx.enter_context(tc.tile_pool(name="abpool", bufs=NQ))
        qpool = ctx.enter_context(tc.tile_pool(name="qpool", bufs=2))
        mpool = ctx.enter_context(tc.tile_pool(name="mpool", bufs=4))
        mpool2 = ctx.enter_context(tc.tile_pool(name="mpool2", bufs=2))
        opool = ctx.enter_context(tc.tile_pool(name="opool", bufs=5))
        ps_acc = ctx.enter_context(tc.tile_pool(name="ps_acc", bufs=3, space="PSUM"))
        ps_log = ctx.enter_context(tc.tile_pool(name="ps_log", bufs=1, space="PSUM"))
        ps_y = ctx.enter_context(tc.tile_pool(name="ps_y", bufs=4, space="PSUM"))

        # ---- persistent SBUF tiles -------------------------------------
        xT0_sb = persist.tile([128, KD, G0], BF, tag="xT0")
        xT1_sb = persist.tile([128, KD, G1], BF, tag="xT1")
        x80_sb = persist.tile([128, KD, G0], E4, tag="x80")
        x81_sb = persist.tile([128, KD, G1], E4, tag="x81")
        w1a_sb = persist.tile([128, KD, H], E4, tag="w1a8")
        w1c_sb = persist.tile([128, KD, H], E4, tag="w1c8")
        wf_sb = persist.tile([128, KD, D], BF, tag="wf")
        m1h0_sb = persist.tile([128, KD, 512], BF, tag="m1h0")
        m1h1_sb = persist.tile([128, KD, 512], BF, tag="m1h1")
        m1_sb = [m1h0_sb, m1h1_sb]
        w2r_sb = persist.tile([128, MH, 128], E4, tag="w2r8")
        id_sb = persist.tile([128, 128], BF, tag="idd")

        # ---- input DMAs: all host-packed contiguous [128, bytes] -------
        # The fp8 x half and the fp8 gating weights land first on separate
        # queues so phase A can start ~10 us in.
        # scalar issues only the three small early fp8 transfers so its
        # engine never blocks on DGE-ring backpressure (evictions follow
        # in its stream); sync carries everything else in deadline order.
        nc.scalar.dma_start(out=w1a_sb, in_=w1a8[:, :])
        nc.scalar.dma_start(out=x81_sb, in_=x81[:, :])
        nc.scalar.dma_start(out=w1c_sb, in_=w1c8[:, :])
        nc.sync.dma_start(out=x80_sb, in_=x80[:, :])
        nc.sync.dma_start(out=xT0_sb, in_=xT0[:, :])
        nc.sync.dma_start(out=wf_sb[:, :, 0:512], in_=wfA[:, :])
        nc.sync.dma_start(out=wf_sb[:, :, 512:D], in_=wfB[:, :])
        nc.sync.dma_start(out=xT1_sb, in_=xT1[:, :])
        nc.sync.dma_start(out=m1_sb[0], in_=m1h0[:, :])
        nc.sync.dma_start(out=m1_sb[1], in_=m1h1[:, :])
        nc.sync.dma_start(out=w2r_sb, in_=w2r8[:, :])
        nc.sync.dma_start(out=id_sb, in_=idd[:, :])
        if use_b1:
            b1_sb = persist.tile([128, MH], FP32, tag="b1")
            nc.sync.dma_start(out=b1_sb, in_=b1r[:, :])
        if use_ub:
            ub_sb = persist.tile([128, MD], FP32, tag="ub")
            nc.sync.dma_start(out=ub_sb, in_=ubr[:, :])
        if use_b2:
            b2_sb = persist.tile([128, 1], FP32, tag="b2")
            nc.sync.dma_start(out=b2_sb, in_=b2h[:, :])
        if use_gamma_beta:
            gam_sb = persist.tile([128, D], FP32, tag="gam")
            nc.sync.dma_start(out=gam_sb, in_=gam.partition_broadcast(128))
            bet_sb = persist.tile([128, D], FP32, tag="bet")
            nc.sync.dma_start(out=bet_sb, in_=bet.partition_broadcast(128))
        if use_merge_b:
            mb_sb = persist.tile([128, D], FP32, tag="mb")
            nc.sync.dma_start(out=mb_sb, in_=mbt.partition_broadcast(128))

        magic_sb = persist.tile([128, 1], I32, tag="magic")
        nc.vector.memset(magic_sb, 0x5F3759DF)
        one_i = persist.tile([128, 1], I32, tag="onei")
        nc.vector.memset(one_i, 1)

        # quarter -> (bf16 x tile, fp8 x tile, shifted-grid base col)
        gmap = [
            (xT0_sb, x80_sb, 0), (xT0_sb, x80_sb, 256),
            (xT1_sb, x81_sb, 0), (xT1_sb, x81_sb, 256),
        ]
        ISCALE = 1.0 / W8SCALE

        aqs, cqs, uqs, tauqs, mmqs = {}, {}, {}, {}, {}

        def emit_A(q):
            xs, x8, base = gmap[q]
            aq = abpool.tile([128, MH, QT], BF, tag="aq")
            aqs[q] = aq
            for m in range(MH):
                ps = ps_acc.tile([128, QG], FP32, tag="acc")
                for kp in range(KD // 2):
                    nc.tensor.matmul(
                        ps[:, 0:QT],
                        w1a_sb[:, 2 * kp:2 * kp + 2, m * 128:(m + 1) * 128],
                        x8[:, 2 * kp:2 * kp + 2, base + HALO:base + HALO + QT],
                        start=(kp == 0), stop=(kp == KD // 2 - 1),
                        perf_mode=DR,
                    )
                if use_b1:
                    nc.scalar.activation(
                        out=aq[:, m, :], in_=ps[:, 0:QT], func=AF.Identity,
                        bias=b1_sb[:, m:m + 1], scale=ISCALE,
                    )
                elif m % 2 == 0:
                    nc.scalar.activation(
                        out=aq[:, m, :], in_=ps[:, 0:QT], func=AF.Identity,
                        bias=0.0, scale=ISCALE,
                    )
                else:
                    nc.vector.tensor_scalar_mul(aq[:, m, :], ps[:, 0:QT], ISCALE)

        def emit_CC(q):
            xs, x8, base = gmap[q]
            cq = abpool.tile([128, MH, QG], BF, tag="cq")
            cqs[q] = cq
            for m in range(MH):
                ps = ps_acc.tile([128, QG], FP32, tag="acc")
                for kp in range(KD // 2):
                    nc.tensor.matmul(
                        ps,
                        w1c_sb[:, 2 * kp:2 * kp + 2, m * 128:(m + 1) * 128],
                        x8[:, 2 * kp:2 * kp + 2, base:base + QG],
                        start=(kp == 0), stop=(kp == KD // 2 - 1),
                        perf_mode=DR,
                    )
                if m % 2 == 0:
                    nc.scalar.activation(
                        out=cq[:, m, :], in_=ps, func=AF.Identity,
                        bias=0.0, scale=ISCALE,
                    )
                else:
                    nc.vector.tensor_scalar_mul(cq[:, m, :], ps, ISCALE)
        def emit_U(q):
            xs, x8, base = gmap[q]
            uq = abpool.tile([128, MD, QG], BF, tag="uq")
            uqs[q] = uq
            for m in range(MD):
                ps = ps_acc.tile([128, QG], FP32, tag="acc")
                for k in range(KD):
                    nc.tensor.matmul(
                        ps, wf_sb[:, k, m * 128:(m + 1) * 128],
                        xs[:, k, base:base + QG],
                        start=(k == 0), stop=(k == KD - 1),
                    )
                if use_ub:
                    nc.scalar.activation(
                        out=uq[:, m, :], in_=ps, func=AF.Identity,
                        bias=ub_sb[:, m:m + 1], scale=1.0,
                    )
                else:
                    nc.scalar.copy(out=uq[:, m, :], in_=ps)

        def emit_C(q):
            aq, cq, uq = aqs[q], cqs[q], uqs[q]
            tauq = qpool.tile([128, W, QT], BF, tag="tauq")
            tauqs[q] = tauq

            def tau_b(w):
                s = tauq[:, w, :]
                return bass.AP(
                    tensor=s.tensor, offset=s.offset,
                    ap=[s.ap[0], [0, MD], s.ap[1]],
                )

            pw = {}
            m01 = None
            for p in range(W // 2):
                hs = mpool2.tile([128, MH, 2, QT], BF, tag="hs")
                for wi in range(2):
                    w = 2 * p + wi
                    o = HALO - 1 - w
                    nc.vector.tensor_add(hs[:, :, wi, :], aq, cq[:, :, o:o + QT])
                hss = mpool2.tile([128, MH, 2, QT], E4, tag="hss")
                nc.scalar.activation(out=hss, in_=hs, func=AF.Silu)
                pl = ps_log.tile([128, 2 * QT], FP32, tag="logit")
                for kp in range(MH // 2):
                    nc.tensor.matmul(
                        pl, w2r_sb[:, 2 * kp:2 * kp + 2, :],
                        hss[:, 2 * kp:2 * kp + 2, :, :],
                        start=(kp == 0), stop=(kp == MH // 2 - 1),
                        perf_mode=DR,
                    )
                # tau = 0.5 + 0.5*tanh(0.5*(logit + b2)); affine done on DVE
                nc.scalar.activation(
                    out=tauq[:, 2 * p:2 * p + 2, :],
                    in_=pl.rearrange("p (a b) -> p a b", a=2),
                    func=AF.Tanh,
                    bias=(b2_sb[:, 0:1] if use_b2 else 0.0),
                    scale=0.5 * ISCALE,
                )
                nc.vector.tensor_scalar(
                    out=tauq[:, 2 * p:2 * p + 2, :],
                    in0=tauq[:, 2 * p:2 * p + 2, :],
                    scalar1=0.5, scalar2=0.5, op0=ALU.mult, op1=ALU.add,
                )
                for wi in range(2):
                    w = 2 * p + wi
                    o = HALO - 1 - w
                    pt = mpool.tile([128, MD, QT], BF, tag="pw")
                    nc.vector.tensor_mul(pt, tau_b(w), uq[:, :, o:o + QT])
                    pw[w] = pt
                if p == 0:
                    m01 = mpool.tile([128, MD, QT], BF, tag="pw")
                    nc.vector.tensor_add(m01, pw[0], pw[1])
            mmq = qpool.tile([128, MD, QT], BF, tag="mmq")
            mmqs[q] = mmq
            nc.vector.tensor_add(pw[3], pw[2], pw[3])
            nc.vector.tensor_add(mmq, m01, pw[3])

        def ln_finalize(q, srow, sqs, ysb, tts):
            # LayerNorm stats for token tiles `tts`; rstd via bit-trick seed
            # + 1 Newton step
            n = len(tts)
            ssum = mpool.tile([128, n], FP32, tag="ssum")
            nc.vector.reduce_sum(out=ssum, in_=srow, axis=AX.X)
            qsum = mpool.tile([128, n], FP32, tag="qsum")
            nc.vector.reduce_sum(out=qsum, in_=sqs, axis=AX.X)
            mean = mpool.tile([128, n], FP32, tag="mean")
            nc.vector.tensor_scalar_mul(mean, ssum, 1.0 / D)
            m2e = mpool.tile([128, n], FP32, tag="m2e")
            nc.vector.scalar_tensor_tensor(   # mean^2 - eps
                out=m2e, in0=mean, scalar=1.0, in1=mean,
                op0=ALU.mult, op1=ALU.mult,
            )
            nc.vector.tensor_scalar_add(m2e, m2e, -EPS)
            veps = mpool.tile([128, n], FP32, tag="veps")
            nc.vector.scalar_tensor_tensor(   # q/D - (mean^2 - eps)
                out=veps, in0=qsum, scalar=1.0 / D, in1=m2e,
                op0=ALU.mult, op1=ALU.subtract,
            )
            rbits = mpool.tile([128, n], I32, tag="rbits")
            nc.vector.tensor_scalar(
                out=rbits, in0=veps.bitcast(I32), scalar1=one_i[:, 0:1],
                scalar2=None, op0=ALU.arith_shift_right,
            )
            nc.vector.tensor_tensor(
                out=rbits, in0=magic_sb.to_broadcast([128, n]), in1=rbits,
                op=ALU.subtract,
            )
            rstd = rbits.bitcast(FP32)
            nt1 = mpool.tile([128, n], FP32, tag="nt1")
            nc.vector.tensor_mul(nt1, rstd, rstd)
            nc.vector.tensor_mul(nt1, nt1, veps)
            nc.vector.tensor_scalar(
                out=nt1, in0=nt1, scalar1=-0.5, scalar2=1.5,
                op0=ALU.mult, op1=ALU.add,
            )
            nc.vector.tensor_mul(rstd, rstd, nt1)
            for i, tt in enumerate(tts):
                tok0 = q * QT + 128 * tt
                nc.vector.tensor_scalar(
                    out=ysb[i], in0=ysb[i], scalar1=mean[:, i:i + 1],
                    scalar2=rstd[:, i:i + 1],
                    op0=ALU.subtract, op1=ALU.mult,
                )
                if use_gamma_beta:
                    nc.vector.tensor_mul(ysb[i], ysb[i], gam_sb)
                    nc.vector.tensor_add(ysb[i], ysb[i], bet_sb)
                nc.sync.dma_start(out=y[tok0:tok0 + 128, :], in_=ysb[i])

        def emit_evict(q, tt, yps, yt, srow, sqs):
            for half in range(2):
                n0 = half * 512
                if use_merge_b:
                    nc.vector.tensor_add(
                        yps[half], yps[half], mb_sb[:, n0:n0 + 512]
                    )
                nc.scalar.activation(
                    out=yt[:, n0:n0 + 512], in_=yps[half], func=AF.Copy,
                    accum_out=srow[:, 0, half:half + 1],
                )
                y2 = mpool2.tile([128, 512], BF, tag="y2")
                nc.vector.scalar_tensor_tensor(
                    out=y2, in0=yt[:, n0:n0 + 512], scalar=1.0,
                    in1=yt[:, n0:n0 + 512], op0=ALU.mult, op1=ALU.mult,
                    accum_out=sqs[:, 0, half:half + 1],
                )

        def emit_D(q, last=False):
            xs, _, base = gmap[q]
            mmq = mmqs[q]
            if not last:
                # m1 matmuls for both token tiles first (they do not depend
                # on the gating chain), then the mm transposes, then evicts.
                ypss, yts, srows, sqss = [], [], [], []
                for tt in range(NT):
                    tcol = base + HALO + tt * 128
                    yp0 = ps_y.tile([128, 512], FP32, tag="y")
                    yp1 = ps_y.tile([128, 512], FP32, tag="y")
                    ypss.append([yp0, yp1])
                    for k in range(KD):
                        for half in range(2):
                            nc.tensor.matmul(
                                ypss[tt][half], xs[:, k, tcol:tcol + 128],
                                m1_sb[half][:, k, :],
                                start=(k == 0), stop=False,
                            )
                for tt in range(NT):
                    for m in range(MD):
                        half, j0 = m // 4, (m % 4) * 128
                        nc.tensor.matmul(
                            ypss[tt][half][:, j0:j0 + 128],
                            mmq[:, m, tt * 128:tt * 128 + 128], id_sb,
                            start=False, stop=(m % 4 == 3),
                            skip_group_check=True,
                        )
                for tt in range(NT):
                    yt = opool.tile([128, D], BF, tag="ysb")
                    srow = mpool.tile([128, 1, 2], FP32, tag="srow")
                    sqs = mpool.tile([128, 1, 2], FP32, tag="sqs")
                    yts.append(yt)
                    srows.append(srow)
                    sqss.append(sqs)
                    emit_evict(q, tt, ypss[tt], yt, srow, sqs)
                for tt in range(NT):
                    ln_finalize(q, srows[tt], sqss[tt], [yts[tt]], [tt])
            else:
                # last quarter: token tiles fully sequential so the tail
                # chain after the final matmul is as short as possible
                for tt in range(NT):
                    tcol = base + HALO + tt * 128
                    yt = opool.tile([128, D], BF, tag="ysb")
                    srow = mpool.tile([128, 1, 2], FP32, tag="srow")
                    sqs = mpool.tile([128, 1, 2], FP32, tag="sqs")
                    yp0 = ps_y.tile([128, 512], FP32, tag="y")
                    yp1 = ps_y.tile([128, 512], FP32, tag="y")
                    yps = [yp0, yp1]
                    for k in range(KD):
                        for half in range(2):
                            nc.tensor.matmul(
                                yps[half], xs[:, k, tcol:tcol + 128],
                                m1_sb[half][:, k, :],
                                start=(k == 0), stop=False,
                            )
                    for m in range(MD):
                        half, j0 = m // 4, (m % 4) * 128
                        nc.tensor.matmul(
                            yps[half][:, j0:j0 + 128],
                            mmq[:, m, tt * 128:tt * 128 + 128], id_sb,
                            start=False, stop=(m % 4 == 3),
                            skip_group_check=True,
                        )
                    emit_evict(q, tt, yps, yt, srow, sqs)
                    ln_finalize(q, srow, sqs, [yt], [tt])

        # software-pipelined emission: a+c for all quarters first (they
        # only need the small fp8 inputs, so the PE starts ~10 us in and
        # stays busy while the bf16 x/weights stream); D lags C by one
        # quarter.  Each phase gets a monotone sim-time floor
        # (tile_wait_until) so the list scheduler cannot hoist a
        # later-phase instruction (whose input DMA is still in flight on
        # hardware) ahead of ready work in the in-order engine queues --
        # without the floors it parked a U-phase matmul mid-AC, stalling
        # the PE ~8 us on the xT0 transfer.
        phases = [
            (lambda q=q: emit_A(q), 0.001 * q) for q in range(NQ)
        ] + [
            (lambda q=q: emit_CC(q), 0.004 + 0.001 * q) for q in range(NQ)
        ] + [
            (lambda: emit_U(0), 0.008),
            (lambda: emit_U(1), 0.010),
            (lambda: emit_C(0), 0.010),
            (lambda: emit_U(2), 0.012),
            (lambda: emit_C(1), 0.012),
            (lambda: emit_D(0), 0.014),
            (lambda: emit_U(3), 0.016),
            (lambda: emit_C(2), 0.016),
            (lambda: emit_D(1), 0.018),
            (lambda: emit_C(3), 0.020),
            (lambda: emit_D(2), 0.020),
            (lambda: emit_D(3, last=True), 0.022),
        ]
        for fn, floor_ms in phases:
            with tc.tile_wait_until(floor_ms):
                fn()
    nc.compile()
    return nc


_CACHE: dict = {}


def _get_nc(*flags):
    if flags not in _CACHE:
        _CACHE[flags] = build_nc(*flags)
    return _CACHE[flags]


def _pack(a):
    # [D, F] -> [128, KD*F] in the SBUF layout (partition = d % 128 within
    # each 128-row K-chunk)
    d, f = a.shape
    return np.ascontiguousarray(
        a.reshape(d // 128, 128, f).transpose(1, 0, 2).reshape(128, -1)
    )


def kernel(x, w1, b1, w2, b2, wv_w, wv_b, merge_w, merge_b, gamma, beta):
    x = np.asarray(x, dtype=np.float32)
    w1 = np.asarray(w1, dtype=np.float32)
    b1 = np.asarray(b1, dtype=np.float32)
    w2 = np.asarray(w2, dtype=np.float32)
    b2 = np.asarray(b2, dtype=np.float32)
    wv_w = np.asarray(wv_w, dtype=np.float32)
    wv_b = np.asarray(wv_b, dtype=np.float32)
    merge_w = np.asarray(merge_w, dtype=np.float32)
    merge_b = np.asarray(merge_b, dtype=np.float32)
    gamma = np.asarray(gamma, dtype=np.float32)
    beta = np.asarray(beta, dtype=np.float32)

    m2 = merge_w[D:]
    wfuse = wv_w @ m2
    ubias = wv_b @ m2
    use_gamma_beta = not (np.all(gamma == 1.0) and np.all(beta == 0.0))
    use_merge_b = bool(np.any(merge_b != 0.0))
    use_b1 = bool(np.any(b1 != 0.0))
    use_b2 = bool(np.any(b2 != 0.0))
    use_ub = bool(np.any(ubias != 0.0))
    nc = _get_nc(use_gamma_beta, use_merge_b, use_b1, use_b2, use_ub)

    shared = {
        "w1a8": _pack((W8SCALE * w1[:D]).astype(F8)),
        "w1c8": _pack((W8SCALE * w1[D:]).astype(F8)),
        "wfA": _pack(wfuse[:, 0:512].astype(BF16)),
        "wfB": _pack(wfuse[:, 512:D].astype(BF16)),
        "m1h0": _pack(merge_w[:D, 0:512].astype(BF16)),
        "m1h1": _pack(merge_w[:D, 512:D].astype(BF16)),
        "w2r8": _pack(
            np.ascontiguousarray(
                np.broadcast_to(
                    (W8SCALE * w2).reshape(H, 1), (H, 128)
                )
            ).astype(F8)
        ),
        "idd": np.eye(128, dtype=np.float32).astype(BF16),
    }
    if use_b1:
        shared["b1r"] = np.ascontiguousarray(b1.reshape(MH, 128).T)
    if use_ub:
        shared["ubr"] = np.ascontiguousarray(ubias.reshape(MD, 128).T)
    if use_b2:
        shared["b2h"] = np.full((128, 1), 0.5 * float(b2[0]), np.float32)
    if use_gamma_beta:
        shared["gam"] = gamma.reshape(1, D)
        shared["bet"] = beta.reshape(1, D)
    if use_merge_b:
        shared["mbt"] = merge_b.reshape(1, D)

    x2 = x.reshape(B * T, D)
    in_maps = []
    for c in range(NCORES):
        t0 = c * NTOK
        xs = np.zeros((GRID, D), np.float32)
        xs[HALO:] = x2[t0:t0 + NTOK]
        if t0 % T != 0:  # halo stays inside the same batch element
            xs[:HALO] = x2[t0 - HALO:t0]
        xt = np.ascontiguousarray(xs.T).astype(BF16)
        x8full = xt.astype(np.float32).astype(F8)
        m = dict(shared)
        m["xT0"] = _pack(xt[:, 0:G0])
        m["xT1"] = _pack(xt[:, 512:GRID])
        m["x80"] = _pack(x8full[:, 0:G0])
        m["x81"] = _pack(x8full[:, 512:GRID])
        in_maps.append(m)

    res = run_bass_kernel_spmd(nc, in_maps, core_ids=list(range(NCORES)))
    out = np.concatenate(
        [r["y"].astype(np.float32) for r in res.results], axis=0
    )
    return out.reshape(B, T, D)
